# revision 7
# baseline (speedup 1.0000x reference)
"""Biaffine edge attention on 8 Trainium2 NeuronCores.

Math (per batch b):
    out[i,o] = head[i,:] @ U @ dep[o,:] + head[i,:]@wh + dep[o,:]@wd + b
with head/dep [S=2048, D=256], U [D,D], edge_W = [wh | wd] (each [D]).

Sharding: pure data-parallel over batch B=8 -> one batch per core,
U / edge_W / edge_b replicated. No collectives.

Per-core kernel:
    ATf[e,i] = sum_d U[d,e] * headT[d,i] + wd[e]      (so the dep-side
               rank-1 term ds[o] rides the e-contraction for free)
    hs[i]    = sum_d headT[d,i] * wh[d] + b           (per-partition bias)
    out[i,o] = sum_e ATf[e,i] * depT[e,o]  + hs[i]
head and dep are transposed on-chip with PE transposes; the hs[i]+b add is
fused into the PSUM->SBUF epilogue copy (ACT/DVE bias add).

Matmuls run as float32r (fp32 bits, 1 cycle/row moving rate for moving
dim >= 256, vs 4 cycles/row for strict fp32) -- switchable to fp32 via
MM_DTYPE below if precision requires.
"""

import numpy as np

import concourse.bass as bass
import concourse.tile as tile
from concourse import bacc, mybir
from concourse.bass_utils import run_bass_kernel_spmd

B, S, D = 8, 2048, 256
P = 128          # partitions
OC = 512         # output free-dim chunk (one PSUM bank of fp32)
NI = S // P      # 16 row blocks
NO = S // OC     # 4 output column chunks
ND = D // P      # 2 contraction chunks
F32 = mybir.dt.float32

# 'f32r' (fast, ~fp32 storage w/ reduced-precision multiply path) or 'f32'.
# FP32r matmul inputs must be *rounded to f32r by their producer op*, so the
# SBUF tensors feeding matmuls are allocated with this dtype and written by
# compute ops (DVE copy / ACT activation), never directly by DMA.
MM_DTYPE = mybir.dt.float32r


def build_nc():
    nc = bacc.Bacc("TRN2", target_bir_lowering=False, debug=False, num_devices=B)

    head_d = nc.dram_tensor("head", [S, D], F32, kind="ExternalInput")
    dep_d = nc.dram_tensor("dep", [S, D], F32, kind="ExternalInput")
    u_d = nc.dram_tensor("U", [D, D], F32, kind="ExternalInput")
    whr_d = nc.dram_tensor("wh_rep", [P, D], F32, kind="ExternalInput")
    wdT_d = nc.dram_tensor("wdT", [P, ND], F32, kind="ExternalInput")
    b128_d = nc.dram_tensor("b128", [P, 1], F32, kind="ExternalInput")
    eye_d = nc.dram_tensor("eye", [P, P], F32, kind="ExternalInput")
    out_d = nc.dram_tensor("out", [S, S], F32, kind="ExternalOutput")

    Ident = mybir.ActivationFunctionType.Identity

    with tile.TileContext(nc) as tc:
        with (
            tc.tile_pool(name="const", bufs=1) as cpool,
            tc.tile_pool(name="persist", bufs=1) as ppool,
            tc.tile_pool(name="stage", bufs=4) as stage,
            tc.tile_pool(name="outbuf", bufs=4) as outbuf,
            tc.tile_pool(name="ps_t", bufs=2, space=bass.MemorySpace.PSUM) as ps_t,
            tc.tile_pool(name="ps_mm", bufs=4, space=bass.MemorySpace.PSUM) as ps_mm,
        ):
            # ---- constants ----
            eye = cpool.tile([P, P], F32, name="eye", tag="eye")
            nc.sync.dma_start(eye[:], eye_d[:])
            u_sb = []
            for dc in range(ND):
                u_stg = cpool.tile([P, D], F32, name=f"ustg{dc}", tag=f"ustg{dc}")
                nc.sync.dma_start(u_stg[:], u_d[dc * P:(dc + 1) * P, :])
                u_t = cpool.tile([P, D], MM_DTYPE, name=f"u{dc}", tag=f"u{dc}")
                nc.vector.tensor_copy(u_t[:], u_stg[:])
                u_sb.append(u_t)
            wh_rep = cpool.tile([P, D], F32, name="wh_rep", tag="wh_rep")
            nc.sync.dma_start(wh_rep[:], whr_d[:])
            wdT = cpool.tile([P, ND], F32, name="wdT", tag="wdT")
            nc.sync.dma_start(wdT[:], wdT_d[:])
            b128 = cpool.tile([P, 1], F32, name="b128", tag="b128")
            nc.sync.dma_start(b128[:], b128_d[:])

            # ---- persistent SBUF tensors ----
            headT = [ppool.tile([P, S], MM_DTYPE, name=f"headT{dc}", tag=f"headT{dc}") for dc in range(ND)]
            depT = [ppool.tile([P, S], MM_DTYPE, name=f"depT{dc}", tag=f"depT{dc}") for dc in range(ND)]
            atf = [ppool.tile([P, S], MM_DTYPE, name=f"atf{eb}", tag=f"atf{eb}") for eb in range(ND)]
            hs_col = ppool.tile([P, NI], F32, name="hs_col", tag="hs_col")
            hs_colb = ppool.tile([P, NI], F32, name="hs_colb", tag="hs_colb")

            # ---- load + transpose head and dep ----
            # For head, also hs[i]+b = reduce(head_tile * wh_rep) + b on DVE.
            def load_transposed(src_dram, dstT, compute_hs):
                for ib in range(NI):
                    nat = stage.tile([P, D], F32, name="nat", tag="nat")
                    nc.sync.dma_start(nat[:], src_dram[ib * P:(ib + 1) * P, :])
                    for dc in range(ND):
                        pst = ps_t.tile([P, P], F32, name="pst", tag="pst")
                        nc.tensor.transpose(
                            pst[:], nat[:, dc * P:(dc + 1) * P], eye[:]
                        )
                        nc.vector.tensor_copy(
                            dstT[dc][:, ib * P:(ib + 1) * P], pst[:]
                        )
                    if compute_hs:
                        ttr = stage.tile([P, D], F32, name="ttr", tag="ttr")
                        nc.vector.tensor_mul(ttr[:], nat[:], wh_rep[:])
                        nc.vector.reduce_sum(
                            hs_col[:, ib:ib + 1], ttr[:],
                            axis=mybir.AxisListType.X,
                        )

            load_transposed(head_d, headT, True)
            # hs_colb = hs + b (single ACT pass; used as the epilogue bias)
            nc.scalar.activation(hs_colb[:], hs_col[:], Ident, bias=b128[:, 0:1])
            load_transposed(dep_d, depT, False)

            # ---- ATf[e,i] = sum_d U[d,e]*headT[d,i], +wd[e] on the copy ----
            for eb in range(ND):
                for ic in range(NO):
                    pa = ps_mm.tile([P, OC], F32, name="psmm", tag="psmm")
                    for dc in range(ND):
                        nc.tensor.matmul(
                            pa[:],
                            u_sb[dc][:, eb * P:(eb + 1) * P],
                            headT[dc][:, ic * OC:(ic + 1) * OC],
                            start=(dc == 0),
                            stop=(dc == ND - 1),
                        )
                    nc.scalar.activation(
                        atf[eb][:, ic * OC:(ic + 1) * OC], pa[:], Ident,
                        bias=wdT[:, eb:eb + 1],
                    )

            # ---- big matmul + fused epilogue ----
            for ib in range(NI):
                for oc in range(NO):
                    po = ps_mm.tile([P, OC], F32, name="psmm", tag="psmm")
                    for eb in range(ND):
                        nc.tensor.matmul(
                            po[:],
                            atf[eb][:, ib * P:(ib + 1) * P],
                            depT[eb][:, oc * OC:(oc + 1) * OC],
                            start=(eb == 0),
                            stop=(eb == ND - 1),
                        )
                    ot = outbuf.tile([P, OC], F32, name="ot", tag="ot")
                    if (ib * NO + oc) % 2 == 0:
                        nc.scalar.activation(
                            ot[:], po[:], Ident, bias=hs_colb[:, ib:ib + 1]
                        )
                    else:
                        nc.vector.tensor_scalar_add(
                            ot[:], po[:], hs_colb[:, ib:ib + 1]
                        )
                    nc.sync.dma_start(
                        out_d[ib * P:(ib + 1) * P, oc * OC:(oc + 1) * OC], ot[:]
                    )

    nc.finalize()
    return nc


_NC_CACHE = {}


def _get_nc():
    if "nc" not in _NC_CACHE:
        _NC_CACHE["nc"] = build_nc()
    return _NC_CACHE["nc"]


def make_in_maps(head, dep, edge_U, edge_W, edge_b):
    head = np.ascontiguousarray(np.asarray(head, dtype=np.float32))
    dep = np.ascontiguousarray(np.asarray(dep, dtype=np.float32))
    u = np.ascontiguousarray(np.asarray(edge_U, dtype=np.float32))
    w = np.asarray(edge_W, dtype=np.float32).reshape(-1)
    wh, wd = w[:D], w[D:]
    wh_rep = np.ascontiguousarray(np.tile(wh[None, :], (P, 1)))
    wdT = np.ascontiguousarray(wd.reshape(ND, P).T)
    b128 = np.full((P, 1), float(np.asarray(edge_b).reshape(-1)[0]), np.float32)
    eye = np.eye(P, dtype=np.float32)
    return [
        {
            "head": head[b], "dep": dep[b], "U": u,
            "wh_rep": wh_rep, "wdT": wdT, "b128": b128, "eye": eye,
        }
        for b in range(B)
    ]


def kernel(head, dep, edge_U, edge_W, edge_b):
    nc = _get_nc()
    in_maps = make_in_maps(head, dep, edge_U, edge_W, edge_b)
    res = run_bass_kernel_spmd(nc, in_maps, core_ids=list(range(B)))
    return np.stack([res.results[b]["out"] for b in range(B)], axis=0)


# revision 10
# speedup vs baseline: 15.7857x; 15.7857x over previous
"""Biaffine edge attention on 8 Trainium2 NeuronCores.

Math (per batch b):
    out[i,o] = head[i,:] @ U @ dep[o,:] + head[i,:]@wh + dep[o,:]@wd + b
with head/dep [S=2048, D=256], U [D,D], edge_W = [wh | wd] (each [D]).

Sharding: pure data-parallel over batch B=8 -> one batch per core,
U / edge_W / edge_b replicated. No collectives.

Per-core kernel:
    ATf[e,i] = sum_d U[d,e] * headT[d,i] + wd[e]      (the dep-side rank-1
               term ds[o] rides the e-contraction for free)
    hs[i]    = sum_d head[i,d] * wh[d]  + b           (DVE mul+reduce;
               per-partition bias in the epilogue)
    out[i,o] = sum_e ATf[e,i] * depT[e,o]  + hs[i]
head and dep are transposed on-chip with PE transposes (batched into
[128,512] PSUM collect tiles). Matmuls run as float32r (1 cycle/row for
moving dim >= 256 vs 4 for strict fp32 => this is what makes the problem
memory- instead of compute-bound). FP32r matmul inputs must be rounded to
f32r by a compute op, so matmul-feeding SBUF tiles are float32r-typed and
written by DVE/ACT copies, never directly by DMA.

DMA sizing: inputs load as [128,1024] group tiles (4 row-blocks per DMA via
a 3D access pattern), outputs store as [128,1024] tiles -- keeps the SP
sequencer's per-DMA dispatch cost (~0.65us) well below the ~60us of data
movement.
"""

import contextlib

import numpy as np

import concourse.bass as bass
import concourse.tile as tile
from concourse import bacc, mybir
from concourse.bass_utils import run_bass_kernel_spmd

B, S, D = 8, 2048, 256
P = 128          # partitions
OC = 512         # matmul output free-dim chunk (one PSUM bank of fp32)
GB = 4           # row-blocks per input load group
NG = S // (P * GB)   # 4 load groups per input
NI = S // P      # 16 row blocks
NO = S // OC     # 4 output column chunks
ND = D // P      # 2 contraction chunks
F32 = mybir.dt.float32
F32R = mybir.dt.float32r


def build_nc(reps=1):
    """reps>1 wraps the body in a HW For_i loop -- used only for timing."""
    nc = bacc.Bacc("TRN2", target_bir_lowering=False, debug=False, num_devices=B)

    head_d = nc.dram_tensor("head", [S, D], F32, kind="ExternalInput")
    dep_d = nc.dram_tensor("dep", [S, D], F32, kind="ExternalInput")
    u_d = nc.dram_tensor("U", [D, D], F32, kind="ExternalInput")
    whr_d = nc.dram_tensor("wh_rep", [P, GB * D], F32, kind="ExternalInput")
    wdT_d = nc.dram_tensor("wdT", [P, ND], F32, kind="ExternalInput")
    b128_d = nc.dram_tensor("b128", [P, 1], F32, kind="ExternalInput")
    eye_d = nc.dram_tensor("eye", [P, P], F32, kind="ExternalInput")
    out_d = nc.dram_tensor("out", [S, S], F32, kind="ExternalOutput")

    Ident = mybir.ActivationFunctionType.Identity

    with tile.TileContext(nc) as tc:
        with (
            tc.tile_pool(name="const", bufs=1) as cpool,
            tc.tile_pool(name="persist", bufs=1) as ppool,
            tc.tile_pool(name="stage", bufs=3) as stage,
            tc.tile_pool(name="ttrp", bufs=2) as ttrp,
            tc.tile_pool(name="outbuf", bufs=4) as outbuf,
            tc.tile_pool(name="ps_t", bufs=2, space=bass.MemorySpace.PSUM) as ps_t,
            tc.tile_pool(name="ps_mm", bufs=4, space=bass.MemorySpace.PSUM) as ps_mm,
        ):
            # ---- constants ----
            eye = cpool.tile([P, P], F32, name="eye", tag="eye")
            nc.sync.dma_start(eye[:], eye_d[:])
            b128 = cpool.tile([P, 1], F32, name="b128", tag="b128")
            nc.sync.dma_start(b128[:], b128_d[:])
            wh_rep = cpool.tile([P, GB * D], F32, name="wh_rep", tag="wh_rep")
            nc.sync.dma_start(wh_rep[:], whr_d[:])
            wdT = cpool.tile([P, ND], F32, name="wdT", tag="wdT")
            nc.sync.dma_start(wdT[:], wdT_d[:])
            u_sb = []
            for dc in range(ND):
                u_stg = cpool.tile([P, D], F32, name=f"ustg{dc}", tag=f"ustg{dc}")
                nc.sync.dma_start(u_stg[:], u_d[dc * P:(dc + 1) * P, :])
                u_t = cpool.tile([P, D], F32R, name=f"u{dc}", tag=f"u{dc}")
                nc.vector.tensor_copy(u_t[:], u_stg[:])
                u_sb.append(u_t)

            # ---- persistent SBUF tensors ----
            headT = [ppool.tile([P, S], F32R, name=f"headT{dc}", tag=f"headT{dc}")
                     for dc in range(ND)]
            depT = [ppool.tile([P, S], F32R, name=f"depT{dc}", tag=f"depT{dc}")
                    for dc in range(ND)]
            atf = [ppool.tile([P, S], F32R, name=f"atf{eb}", tag=f"atf{eb}")
                   for eb in range(ND)]
            hs_col = ppool.tile([P, NI], F32, name="hs_col", tag="hs_col")
            hs_colb = ppool.tile([P, NI], F32, name="hs_colb", tag="hs_colb")

            def load_group(src_dram, g):
                # [128, GB*D]: free = (block j, d); one DMA, 3D src pattern
                nat = stage.tile([P, GB * D], F32, name="nat", tag="nat")
                src = src_dram[g * GB * P:(g + 1) * GB * P, :]
                src3 = src.rearrange("(j p) d -> p j d", p=P)
                nc.sync.dma_start(nat[:].rearrange("p (j d) -> p j d", d=D), src3)
                return nat

            def transpose_group(nat, dstT, g, eng_off):
                # 8 PE transposes -> two [128,512] PSUM collect tiles -> 2 copies
                for dc in range(ND):
                    pst = ps_t.tile([P, GB * P], F32, name="pst", tag="pst")
                    for j in range(GB):
                        nc.tensor.transpose(
                            pst[:, j * P:(j + 1) * P],
                            nat[:, j * D + dc * P: j * D + dc * P + P],
                            eye[:],
                        )
                    dst = dstT[dc][:, g * GB * P:(g + 1) * GB * P]
                    if (g * ND + dc + eng_off) % 2 == 0:
                        nc.vector.tensor_copy(dst, pst[:])
                    else:
                        nc.scalar.copy(dst, pst[:])

            def body():
                # ---- interleaved loads / transposes / hs / AT ----
                for g in range(NG):
                    nat_h = load_group(head_d, g)
                    nat_p = load_group(dep_d, g)
                    transpose_group(nat_h, headT, g, 0)
                    # hs for this group's 4 blocks: mul + blockwise reduce
                    ttr = ttrp.tile([P, GB * D], F32, name="ttr", tag="ttr")
                    nc.vector.tensor_mul(ttr[:], nat_h[:], wh_rep[:])
                    nc.vector.reduce_sum(
                        hs_col[:, g * GB:(g + 1) * GB],
                        ttr[:].rearrange("p (j d) -> p j d", d=D),
                        axis=mybir.AxisListType.X,
                    )
                    transpose_group(nat_p, depT, g, 1)
                    # ATf chunk ic=g (headT[:, g*512:(g+1)*512] just written)
                    for eb in range(ND):
                        pa = ps_mm.tile([P, OC], F32, name="psmm", tag="psmm")
                        for dc in range(ND):
                            nc.tensor.matmul(
                                pa[:],
                                u_sb[dc][:, eb * P:(eb + 1) * P],
                                headT[dc][:, g * OC:(g + 1) * OC],
                                start=(dc == 0),
                                stop=(dc == ND - 1),
                            )
                        nc.scalar.activation(
                            atf[eb][:, g * OC:(g + 1) * OC], pa[:], Ident,
                            bias=wdT[:, eb:eb + 1],
                        )

                # hs_colb = hs + b (single ACT pass; the epilogue bias)
                nc.scalar.activation(hs_colb[:], hs_col[:], Ident,
                                     bias=b128[:, 0:1])

                # ---- big matmul + fused epilogue, [128,1024] out tiles ----
                for ib in range(NI):
                    for op in range(NO // 2):
                        ot = outbuf.tile([P, 2 * OC], F32, name="ot", tag="ot")
                        for half in range(2):
                            oc = op * 2 + half
                            po = ps_mm.tile([P, OC], F32, name="psmm",
                                            tag="psmm")
                            for eb in range(ND):
                                nc.tensor.matmul(
                                    po[:],
                                    atf[eb][:, ib * P:(ib + 1) * P],
                                    depT[eb][:, oc * OC:(oc + 1) * OC],
                                    start=(eb == 0),
                                    stop=(eb == ND - 1),
                                )
                            dst = ot[:, half * OC:(half + 1) * OC]
                            if (ib + op + half) % 2 == 0:
                                nc.scalar.activation(
                                    dst, po[:], Ident,
                                    bias=hs_colb[:, ib:ib + 1],
                                )
                            else:
                                nc.vector.tensor_scalar_add(
                                    dst, po[:], hs_colb[:, ib:ib + 1]
                                )
                        nc.sync.dma_start(
                            out_d[ib * P:(ib + 1) * P,
                                  op * 2 * OC:(op + 1) * 2 * OC],
                            ot[:],
                        )

            if reps > 1:
                with tc.For_i(0, reps, 1):
                    body()
            else:
                body()

    nc.finalize()
    return nc


_NC_CACHE = {}


def _get_nc(reps=1):
    if reps not in _NC_CACHE:
        _NC_CACHE[reps] = build_nc(reps)
    return _NC_CACHE[reps]


def make_in_maps(head, dep, edge_U, edge_W, edge_b):
    head = np.ascontiguousarray(np.asarray(head, dtype=np.float32))
    dep = np.ascontiguousarray(np.asarray(dep, dtype=np.float32))
    u = np.ascontiguousarray(np.asarray(edge_U, dtype=np.float32))
    w = np.asarray(edge_W, dtype=np.float32).reshape(-1)
    wh, wd = w[:D], w[D:]
    wh_rep = np.ascontiguousarray(np.tile(wh[None, :], (P, GB)))
    wdT = np.ascontiguousarray(wd.reshape(ND, P).T)
    b128 = np.full((P, 1), float(np.asarray(edge_b).reshape(-1)[0]), np.float32)
    eye = np.eye(P, dtype=np.float32)
    return [
        {
            "head": head[b], "dep": dep[b], "U": u,
            "wh_rep": wh_rep, "wdT": wdT, "b128": b128, "eye": eye,
        }
        for b in range(B)
    ]


def kernel(head, dep, edge_U, edge_W, edge_b):
    nc = _get_nc()
    in_maps = make_in_maps(head, dep, edge_U, edge_W, edge_b)
    res = run_bass_kernel_spmd(nc, in_maps, core_ids=list(range(B)))
    return np.stack([res.results[b]["out"] for b in range(B)], axis=0)
